# revision 8
# baseline (speedup 1.0000x reference)
"""Causal self-attention (RoPE) Trainium2 kernel.

Sharding: 2 batches x 16 heads = 32 (b,h) units over 8 cores -> each core
handles 1 batch x 4 heads. Column-parallel QKV + row-parallel output
projection; host sums the 4 partial outputs per batch (fp16 partials).

All matmul operands are fp16 (1 cycle/col on the PE; fp32 PSUM accum).

Per-core structure:

  V: contraction(cc)-outer over two groups of 8 PSUM tiles so the PE
     starts as soon as the first x chunk lands (x DMA overlaps V).
  QK(head h): Q^T,K^T in [d, t] layout + RoPE, pipelined with ATT(h-1).
  ATT per (h, q-tile 512): S^T[k,q] = K^T_chunk^T @ Q^T. Causal diagonal
     chunks are width-sliced (512/384/256/128) instead of masked at full
     width; only the triangle block needs the additive mask before exp.
     y^T[d,q] accumulates A@V in PSUM; denominator via a ones-column
     matmul over the same (sliced) A tiles.
  PROJ interleaved with ATT(3) per q-tile: out rows accumulate into a
     [128, 2048] fp16 SBUF tile, one 1 MB DMA per 128 output rows so the
     store stream overlaps the tail of attention.
"""

import sys

if "/opt/trn_rl_repo" not in sys.path:
    sys.path.insert(0, "/opt/trn_rl_repo")

import numpy as np

import concourse.bass as bass
import concourse.tile as tile
from concourse import bacc, mybir
from concourse.bass_utils import run_bass_kernel_spmd

F32 = mybir.dt.float32
F16 = mybir.dt.float16

B, T, C = 2, 2048, 2048
NH, HD = 16, 128
NHL = 4            # heads per core
D_LOC = NHL * HD   # 512 local head dims
N_CORES = 8
SCALE = 1.0 / float(np.sqrt(HD))
NEG = -30000.0     # big enough: exp((S+NEG)*SCALE) == 0 for |S| < ~1000

CC = C // 128      # 16 contraction chunks
KC = T // 128      # 16 key chunks
QT = 512           # q tile
NQT = T // QT      # 4 q tiles

_compiled = None


def _build():
    nc = bacc.Bacc("TRN2", target_bir_lowering=False, debug=False)

    xT_d = nc.dram_tensor("xT", [C, T], F16, kind="ExternalInput")
    wq_d = nc.dram_tensor("wq", [NHL, 128, CC, 128], F16, kind="ExternalInput")
    wk_d = nc.dram_tensor("wk", [NHL, 128, CC, 128], F16, kind="ExternalInput")
    wv_d = nc.dram_tensor("wv", [128, CC, D_LOC], F16, kind="ExternalInput")
    w2_d = nc.dram_tensor("w2", [128, NHL, C], F16, kind="ExternalInput")
    cos2_d = nc.dram_tensor("cos2", [128, T], F16, kind="ExternalInput")
    sin2s_d = nc.dram_tensor("sin2s", [128, T], F16, kind="ExternalInput")
    masks_d = nc.dram_tensor("masks", [128, QT], F16, kind="ExternalInput")
    out_d = nc.dram_tensor("out", [T, C], F16, kind="ExternalOutput")

    swap_mask = list(range(16, 32)) + list(range(16))

    with tile.TileContext(nc) as tc, \
         tc.tile_pool(name="persist", bufs=1) as persist, \
         tc.tile_pool(name="px", bufs=1) as px, \
         tc.tile_pool(name="pw", bufs=3) as pw, \
         tc.tile_pool(name="rope", bufs=2) as prope, \
         tc.tile_pool(name="att", bufs=6) as patt, \
         tc.tile_pool(name="nrm", bufs=1) as pnrm, \
         tc.tile_pool(name="outp", bufs=2) as pout, \
         tc.tile_pool(name="psqk", bufs=2, space="PSUM") as psqk, \
         tc.tile_pool(name="pst", bufs=3, space="PSUM") as pst, \
         tc.tile_pool(name="psy", bufs=2, space="PSUM") as psy, \
         tc.tile_pool(name="psd", bufs=1, space="PSUM") as psd:
        # persistent tiles
        qkT = persist.tile([128, 8, T], F16, tag="qkT")
        v_sb = persist.tile([128, KC, D_LOC], F16, tag="vsb")
        yT = persist.tile([128, NHL, T], F16, tag="yT")
        masks_sb = persist.tile([128, QT], F16, tag="masks")
        cos2 = persist.tile([128, T], F16, tag="cos2")
        sin2s = persist.tile([128, T], F16, tag="sin2s")
        w2_sb = persist.tile([128, NHL, C], F16, tag="w2")
        ones_sb = persist.tile([128, 1], F16, tag="ones")
        nc.vector.memset(ones_sb, 1.0)

        # ---- DMA emission in first-use order; x chunks issue from the
        # (startup-idle) scalar HWDGE queue in parallel with wv on sync,
        # halving the serialized DMA_DIRECT2D issue latency ----
        xs = px.tile([128, CC, T], F16, tag="xs")
        wv_sb = pw.tile([128, CC, D_LOC], F16, tag="wv", bufs=1)
        # Everything V-critical on the fast sync queue: first x chunks, then
        # wv (needed with chunk 0), then the rest of x. Scalar HWDGE
        # transfers are slow — only late-use tensors go there.
        for cc in range(4):
            nc.sync.dma_start(
                out=xs[:, cc, :],
                in_=xT_d.ap()[cc * 128:(cc + 1) * 128, :],
            )
        for wc in range(4):
            nc.sync.dma_start(
                out=wv_sb[:, 4 * wc:4 * (wc + 1), :],
                in_=wv_d.ap()[:, 4 * wc:4 * (wc + 1), :],
            )
        for cc in range(4, CC):
            nc.sync.dma_start(
                out=xs[:, cc, :],
                in_=xT_d.ap()[cc * 128:(cc + 1) * 128, :],
            )
        nc.scalar.dma_start(out=cos2, in_=cos2_d.ap())
        nc.scalar.dma_start(out=sin2s, in_=sin2s_d.ap())
        nc.scalar.dma_start(out=masks_sb, in_=masks_d.ap())
        nc.scalar.dma_start(out=w2_sb, in_=w2_d.ap())

        def emit_v():
            # contraction-outer over groups of 8 tch tiles: PE consumes x
            # chunk cc as soon as it lands; 8 live PSUM tiles span all pools.
            for grp in range(2):
                pvs = []
                for j, (pool, tg) in enumerate(
                        [(psqk, "qk"), (psqk, "qk"), (pst, "st"), (pst, "st"),
                         (pst, "st"), (psy, "y"), (psy, "y"), (psd, "d")]):
                    pvs.append(pool.tile([128, D_LOC], F32, tag=tg,
                                         name=f"pv{grp}_{j}"))
                for cc in range(CC):
                    for j in range(8):
                        tch = grp * 8 + j
                        nc.tensor.matmul(
                            pvs[j],
                            xs[:, cc, tch * 128:(tch + 1) * 128],
                            wv_sb[:, cc, :],
                            start=(cc == 0), stop=(cc == CC - 1),
                        )
                for j in range(8):
                    nc.scalar.copy(v_sb[:, grp * 8 + j, :], pvs[j])

        def emit_qk(h):
            # jc pair (Q head h, K head h)
            for jc in (h, 4 + h):
                w_src = (wq_d if jc < 4 else wk_d).ap()[jc % 4]
                w_sb = pw.tile([128, CC, 128], F16, tag="w",
                               name=f"w_sb{jc}")
                nc.sync.dma_start(out=w_sb, in_=w_src)
                for tt in range(NQT):
                    gt0 = tt * QT
                    ps = psqk.tile([128, QT], F32, tag="qk",
                                   name=f"psqk{jc}_{tt}")
                    for cc in range(CC):
                        nc.tensor.matmul(
                            ps, w_sb[:, cc, :],
                            xs[:, cc, gt0:gt0 + QT],
                            start=(cc == 0), stop=(cc == CC - 1),
                        )
                    u = prope.tile([128, QT], F16, tag="u", name=f"u{jc}{tt}")
                    v = prope.tile([128, QT], F16, tag="v", name=f"v{jc}{tt}")
                    w = prope.tile([128, QT], F16, tag="w", name=f"w{jc}{tt}")
                    nc.vector.tensor_mul(u, ps, cos2[:, gt0:gt0 + QT])
                    nc.vector.tensor_mul(v, ps, sin2s[:, gt0:gt0 + QT])
                    nc.vector.stream_shuffle(w, v, swap_mask)
                    nc.vector.tensor_add(qkT[:, jc, gt0:gt0 + QT], u, w)

        def emit_proj_qt(qt, ci):
            # out rows [qc*128, (qc+1)*128) for qc in this q-tile; 4 head
            # matmuls per 512-col block accumulate in PSUM (psqk pool: idle
            # during ATT(3)), copies alternate scalar/vector engines, one
            # 1 MB DMA per 128 rows.
            for qc in range(4 * qt, 4 * qt + 4):
                osb = pout.tile([128, C], F16, tag="o", name=f"osb{qc}")
                for ct in range(C // QT):
                    ops = psqk.tile([128, QT], F32, tag="qk",
                                    name=f"ops{qc}{ct}")
                    for h in range(NHL):
                        nc.tensor.matmul(
                            ops,
                            yT[:, h, qc * 128:(qc + 1) * 128],
                            w2_sb[:, h, ct * QT:(ct + 1) * QT],
                            start=(h == 0), stop=(h == NHL - 1),
                        )
                    if ci[0] % 2 == 0:
                        nc.scalar.copy(osb[:, ct * QT:(ct + 1) * QT], ops)
                    else:
                        nc.vector.tensor_copy(osb[:, ct * QT:(ct + 1) * QT],
                                              ops)
                    ci[0] += 1
                    if qc == T // 128 - 1:
                        # last row block: per-column-block DMA so the final
                        # store overlaps the remaining copies (tail trim)
                        nc.sync.dma_start(
                            out=out_d.ap()[qc * 128:(qc + 1) * 128,
                                           ct * QT:(ct + 1) * QT],
                            in_=osb[:, ct * QT:(ct + 1) * QT],
                        )
                if qc != T // 128 - 1:
                    nc.sync.dma_start(
                        out=out_d.ap()[qc * 128:(qc + 1) * 128, :],
                        in_=osb,
                    )

        def emit_att(h, with_proj=False):
            ci = [0]
            for qt in range(NQT):
                q0 = qt * QT
                nkc = 4 * qt + 4  # valid k chunks (causal)
                qT_ap = qkT[:, h, q0:q0 + QT]
                yps = psy.tile([128, QT], F32, tag="y", name=f"yps{h}{qt}")
                dps = psd.tile([1, QT], F32, tag="d", name=f"dps{h}{qt}")
                a_tiles = [None] * nkc

                def emit_st_exp(kc):
                    # diagonal chunks only need q >= kc*128: slice width
                    o = kc - 4 * qt
                    f0 = max(o, 0) * 128  # in-tile q offset
                    sps = pst.tile([128, QT], F32, tag="st",
                                   name=f"sps{h}{qt}{kc}")
                    nc.tensor.matmul(
                        sps[:, f0:], qkT[:, 4 + h, kc * 128:(kc + 1) * 128],
                        qT_ap[:, f0:], start=True, stop=True,
                    )
                    if o >= 0:
                        nc.vector.tensor_add(
                            sps[:, f0:], sps[:, f0:],
                            masks_sb[:, :QT - f0])
                    a = patt.tile([128, QT], F16, tag="a", name=f"a{h}{qt}{kc}")
                    nc.scalar.activation(
                        a[:, f0:], sps[:, f0:],
                        mybir.ActivationFunctionType.Exp,
                        scale=SCALE,
                    )
                    a_tiles[kc] = (a, f0)

                emit_st_exp(0)
                if nkc > 1:
                    emit_st_exp(1)
                for kc in range(nkc):
                    if kc + 2 < nkc:
                        emit_st_exp(kc + 2)
                    a, f0 = a_tiles[kc]
                    nc.tensor.matmul(
                        yps[:, f0:], v_sb[:, kc, h * HD:(h + 1) * HD],
                        a[:, f0:],
                        start=(kc == 0), stop=(kc == nkc - 1),
                        skip_group_check=True,
                    )
                    nc.tensor.matmul(
                        dps[:, f0:], ones_sb, a[:, f0:],
                        start=(kc == 0), stop=(kc == nkc - 1),
                        skip_group_check=True,
                    )
                rinv = pnrm.tile([1, QT], F32, tag="rinv", name=f"ri{h}{qt}")
                nc.vector.reciprocal_approx_fast(rinv, dps)
                rb = pnrm.tile([128, QT], F32, tag="rb", name=f"rb{h}{qt}")
                nc.gpsimd.partition_broadcast(rb, rinv)
                nc.vector.tensor_mul(yT[:, h, q0:q0 + QT], yps, rb)
                if with_proj and qt > 0:
                    emit_proj_qt(qt - 1, ci)
            if with_proj:
                emit_proj_qt(NQT - 1, ci)

        emit_v()
        emit_qk(0)
        for h in range(NHL - 1):
            emit_qk(h + 1)
            emit_att(h)
        emit_att(NHL - 1, with_proj=True)

    nc.compile()
    return nc


def _prep_core_inputs(core, x16, W_attn, W_proj, cos2, sin2s, masks):
    b = core // 4
    g = core % 4
    heads = [g * NHL + i for i in range(NHL)]
    # stream_shuffle permutes within 32-partition blocks only: lay out each
    # block as [re pairs 16b..16b+15 | im pairs 16b..16b+15]
    perm = np.concatenate(
        [np.r_[2 * (16 * blk + np.arange(16)),
               2 * (16 * blk + np.arange(16)) + 1]
         for blk in range(4)]
    )

    xT = np.ascontiguousarray(x16[b].T)

    def qk_blocks(base):
        blocks = []
        for h in heads:
            blk = W_attn[:, base + h * HD: base + (h + 1) * HD][:, perm]
            blocks.append(blk.reshape(CC, 128, HD).transpose(1, 0, 2))
        return np.ascontiguousarray(np.stack(blocks, axis=0)).astype(np.float16)

    wq = qk_blocks(0)
    wk = qk_blocks(C)
    wv = np.concatenate(
        [W_attn[:, 2 * C + h * HD: 2 * C + (h + 1) * HD] for h in heads],
        axis=1,
    )  # (C, D_LOC)
    wv = np.ascontiguousarray(
        wv.reshape(CC, 128, D_LOC).transpose(1, 0, 2)).astype(np.float16)
    w2 = np.ascontiguousarray(
        np.stack([W_proj[h * HD:(h + 1) * HD, :] for h in heads], axis=0)
        .transpose(1, 0, 2)
    ).astype(np.float16)
    return {
        "xT": xT, "wq": wq, "wk": wk, "wv": wv, "w2": w2,
        "cos2": cos2, "sin2s": sin2s, "masks": masks,
    }


def _run(inputs, trace=False):
    global _compiled
    x = np.asarray(inputs["x"], dtype=np.float32)
    W_attn = np.asarray(inputs["W_attn"], dtype=np.float32)
    W_proj = np.asarray(inputs["W_proj"], dtype=np.float32)
    fc = np.asarray(inputs["freqs_cos"], dtype=np.float32)
    fs = np.asarray(inputs["freqs_sin"], dtype=np.float32)

    x16 = x.astype(np.float16)

    cosT = np.ascontiguousarray(fc.T)            # (64, T)
    sinT = np.ascontiguousarray(fs.T)
    # per 32-partition block b: partitions [0:16] carry cos/sin of pairs
    # 16b..16b+15 (re half, +sin), [16:32] the same freqs (im half, -sin)
    cos2 = np.concatenate(
        [np.concatenate([cosT[16 * blk:16 * (blk + 1)]] * 2, axis=0)
         for blk in range(4)], axis=0)           # (128, T)
    sin2s = np.concatenate(
        [np.concatenate([sinT[16 * blk:16 * (blk + 1)],
                         -sinT[16 * blk:16 * (blk + 1)]], axis=0)
         for blk in range(4)], axis=0)
    cos2 = np.ascontiguousarray(cos2).astype(np.float16)
    sin2s = np.ascontiguousarray(sin2s).astype(np.float16)

    # diagonal-chunk mask: within a chunk's valid q window, col j (q =
    # key_base + j): visible iff j >= partition k; cols >= 128 always visible
    ki = np.arange(128)[:, None]
    u = np.arange(QT)[None, :]
    masks = np.ascontiguousarray(
        np.where(u >= ki, 0.0, NEG).astype(np.float16))  # (128, 512)

    if _compiled is None:
        _compiled = _build()
    nc = _compiled

    in_maps = [
        _prep_core_inputs(c, x16, W_attn, W_proj, cos2, sin2s, masks)
        for c in range(N_CORES)
    ]
    res = run_bass_kernel_spmd(
        nc, in_maps, core_ids=list(range(N_CORES)), trace=trace)

    out = np.zeros((B, T, C), dtype=np.float32)
    for c in range(N_CORES):
        out[c // 4] += res.results[c]["out"].astype(np.float32)
    return out, res


def kernel(**inputs) -> np.ndarray:
    out, _ = _run(inputs, trace=False)
    return out


# revision 9
# speedup vs baseline: 1.0381x; 1.0381x over previous
"""Causal self-attention (RoPE) Trainium2 kernel.

Sharding: 2 batches x 16 heads = 32 (b,h) units over 8 cores -> each core
handles 1 batch x 4 heads. Column-parallel QKV + row-parallel output
projection; host sums the 4 partial outputs per batch (fp16 partials).

All matmul operands are fp16 (1 cycle/col on the PE; fp32 PSUM accum).

Per-core structure:

  V: contraction(cc)-outer over two groups of 8 PSUM tiles so the PE
     starts as soon as the first x chunk lands (x DMA overlaps V).
  QK(head h): Q^T,K^T in [d, t] layout + RoPE, pipelined with ATT(h-1).
  ATT per (h, q-tile 512): S^T[k,q] = K^T_chunk^T @ Q^T. Causal diagonal
     chunks are width-sliced (512/384/256/128) instead of masked at full
     width; only the triangle block needs the additive mask before exp.
     y^T[d,q] accumulates A@V in PSUM; denominator via a ones-column
     matmul over the same (sliced) A tiles.
  PROJ interleaved with ATT(3) per q-tile: out rows accumulate into a
     [128, 2048] fp16 SBUF tile, one 1 MB DMA per 128 output rows so the
     store stream overlaps the tail of attention.
"""

import sys

if "/opt/trn_rl_repo" not in sys.path:
    sys.path.insert(0, "/opt/trn_rl_repo")

import numpy as np

import concourse.bass as bass
import concourse.tile as tile
from concourse import bacc, mybir
from concourse.bass_utils import run_bass_kernel_spmd

F32 = mybir.dt.float32
F16 = mybir.dt.float16

B, T, C = 2, 2048, 2048
NH, HD = 16, 128
NHL = 4            # heads per core
D_LOC = NHL * HD   # 512 local head dims
N_CORES = 8
SCALE = 1.0 / float(np.sqrt(HD))
NEG = -30000.0     # big enough: exp((S+NEG)*SCALE) == 0 for |S| < ~1000

CC = C // 128      # 16 contraction chunks
KC = T // 128      # 16 key chunks
QT = 512           # q tile
NQT = T // QT      # 4 q tiles

_compiled = None


def _build():
    nc = bacc.Bacc("TRN2", target_bir_lowering=False, debug=False)

    xT_d = nc.dram_tensor("xT", [C, T], F16, kind="ExternalInput")
    wq_d = nc.dram_tensor("wq", [NHL, 128, CC, 128], F16, kind="ExternalInput")
    wk_d = nc.dram_tensor("wk", [NHL, 128, CC, 128], F16, kind="ExternalInput")
    wv_d = nc.dram_tensor("wv", [128, CC, D_LOC], F16, kind="ExternalInput")
    w2_d = nc.dram_tensor("w2", [128, NHL, C], F16, kind="ExternalInput")
    cos2_d = nc.dram_tensor("cos2", [128, T], F16, kind="ExternalInput")
    sin2s_d = nc.dram_tensor("sin2s", [128, T], F16, kind="ExternalInput")
    masks_d = nc.dram_tensor("masks", [128, QT], F16, kind="ExternalInput")
    out_d = nc.dram_tensor("out", [T, C], F16, kind="ExternalOutput")

    swap_mask = list(range(16, 32)) + list(range(16))

    with tile.TileContext(nc) as tc, \
         tc.tile_pool(name="persist", bufs=1) as persist, \
         tc.tile_pool(name="px", bufs=1) as px, \
         tc.tile_pool(name="pw", bufs=3) as pw, \
         tc.tile_pool(name="rope", bufs=2) as prope, \
         tc.tile_pool(name="att", bufs=6) as patt, \
         tc.tile_pool(name="nrm", bufs=1) as pnrm, \
         tc.tile_pool(name="outp", bufs=2) as pout, \
         tc.tile_pool(name="psqk", bufs=2, space="PSUM") as psqk, \
         tc.tile_pool(name="pst", bufs=3, space="PSUM") as pst, \
         tc.tile_pool(name="psy", bufs=2, space="PSUM") as psy, \
         tc.tile_pool(name="psd", bufs=1, space="PSUM") as psd:
        # persistent tiles
        qkT = persist.tile([128, 8, T], F16, tag="qkT")
        v_sb = persist.tile([128, KC, D_LOC], F16, tag="vsb")
        yT = persist.tile([128, NHL, T], F16, tag="yT")
        masks_sb = persist.tile([128, QT], F16, tag="masks")
        cos2 = persist.tile([128, T], F16, tag="cos2")
        sin2s = persist.tile([128, T], F16, tag="sin2s")
        w2_sb = persist.tile([128, NHL, C], F16, tag="w2")
        ones_sb = persist.tile([128, 1], F16, tag="ones")
        nc.vector.memset(ones_sb, 1.0)

        # ---- DMA emission in first-use order; x chunks issue from the
        # (startup-idle) scalar HWDGE queue in parallel with wv on sync,
        # halving the serialized DMA_DIRECT2D issue latency ----
        xs = px.tile([128, CC, T], F16, tag="xs")
        wv_sb = pw.tile([128, CC, D_LOC], F16, tag="wv", bufs=1)
        # All input DMAs on the sync queue (DMA rings are shared round-robin
        # across issuing engines; a slow transfer on another queue blocks the
        # ring). Interleave wv pieces with x chunks in first-use order.
        for wc in range(4):
            nc.sync.dma_start(
                out=wv_sb[:, 4 * wc:4 * (wc + 1), :],
                in_=wv_d.ap()[:, 4 * wc:4 * (wc + 1), :],
            )
            for cc in range(4 * wc, 4 * wc + 4):
                nc.sync.dma_start(
                    out=xs[:, cc, :],
                    in_=xT_d.ap()[cc * 128:(cc + 1) * 128, :],
                )
        nc.sync.dma_start(out=cos2, in_=cos2_d.ap())
        nc.sync.dma_start(out=sin2s, in_=sin2s_d.ap())
        nc.sync.dma_start(out=masks_sb, in_=masks_d.ap())
        nc.sync.dma_start(out=w2_sb, in_=w2_d.ap())

        def emit_v():
            # contraction-outer over groups of 8 tch tiles: PE consumes x
            # chunk cc as soon as it lands; 8 live PSUM tiles span all pools.
            for grp in range(2):
                pvs = []
                for j, (pool, tg) in enumerate(
                        [(psqk, "qk"), (psqk, "qk"), (pst, "st"), (pst, "st"),
                         (pst, "st"), (psy, "y"), (psy, "y"), (psd, "d")]):
                    pvs.append(pool.tile([128, D_LOC], F32, tag=tg,
                                         name=f"pv{grp}_{j}"))
                for cc in range(CC):
                    for j in range(8):
                        tch = grp * 8 + j
                        nc.tensor.matmul(
                            pvs[j],
                            xs[:, cc, tch * 128:(tch + 1) * 128],
                            wv_sb[:, cc, :],
                            start=(cc == 0), stop=(cc == CC - 1),
                        )
                for j in range(8):
                    nc.scalar.copy(v_sb[:, grp * 8 + j, :], pvs[j])

        def emit_qk(h):
            # jc pair (Q head h, K head h)
            for jc in (h, 4 + h):
                w_src = (wq_d if jc < 4 else wk_d).ap()[jc % 4]
                w_sb = pw.tile([128, CC, 128], F16, tag="w",
                               name=f"w_sb{jc}")
                nc.sync.dma_start(out=w_sb, in_=w_src)
                for tt in range(NQT):
                    gt0 = tt * QT
                    ps = psqk.tile([128, QT], F32, tag="qk",
                                   name=f"psqk{jc}_{tt}")
                    for cc in range(CC):
                        nc.tensor.matmul(
                            ps, w_sb[:, cc, :],
                            xs[:, cc, gt0:gt0 + QT],
                            start=(cc == 0), stop=(cc == CC - 1),
                        )
                    u = prope.tile([128, QT], F16, tag="u", name=f"u{jc}{tt}")
                    v = prope.tile([128, QT], F16, tag="v", name=f"v{jc}{tt}")
                    w = prope.tile([128, QT], F16, tag="w", name=f"w{jc}{tt}")
                    nc.vector.tensor_mul(u, ps, cos2[:, gt0:gt0 + QT])
                    nc.vector.tensor_mul(v, ps, sin2s[:, gt0:gt0 + QT])
                    nc.vector.stream_shuffle(w, v, swap_mask)
                    nc.vector.tensor_add(qkT[:, jc, gt0:gt0 + QT], u, w)

        def emit_proj_qt(qt, ci):
            # out rows [qc*128, (qc+1)*128) for qc in this q-tile; 4 head
            # matmuls per 512-col block accumulate in PSUM (psqk pool: idle
            # during ATT(3)), copies alternate scalar/vector engines, one
            # 1 MB DMA per 128 rows.
            for qc in range(4 * qt, 4 * qt + 4):
                osb = pout.tile([128, C], F16, tag="o", name=f"osb{qc}")
                for ct in range(C // QT):
                    ops = psqk.tile([128, QT], F32, tag="qk",
                                    name=f"ops{qc}{ct}")
                    for h in range(NHL):
                        nc.tensor.matmul(
                            ops,
                            yT[:, h, qc * 128:(qc + 1) * 128],
                            w2_sb[:, h, ct * QT:(ct + 1) * QT],
                            start=(h == 0), stop=(h == NHL - 1),
                        )
                    if ci[0] % 2 == 0:
                        nc.scalar.copy(osb[:, ct * QT:(ct + 1) * QT], ops)
                    else:
                        nc.vector.tensor_copy(osb[:, ct * QT:(ct + 1) * QT],
                                              ops)
                    ci[0] += 1
                    if qc == T // 128 - 1:
                        # last row block: per-column-block DMA so the final
                        # store overlaps the remaining copies (tail trim)
                        nc.sync.dma_start(
                            out=out_d.ap()[qc * 128:(qc + 1) * 128,
                                           ct * QT:(ct + 1) * QT],
                            in_=osb[:, ct * QT:(ct + 1) * QT],
                        )
                if qc != T // 128 - 1:
                    nc.sync.dma_start(
                        out=out_d.ap()[qc * 128:(qc + 1) * 128, :],
                        in_=osb,
                    )

        def emit_att(h, with_proj=False):
            ci = [0]
            for qt in range(NQT):
                q0 = qt * QT
                nkc = 4 * qt + 4  # valid k chunks (causal)
                qT_ap = qkT[:, h, q0:q0 + QT]
                yps = psy.tile([128, QT], F32, tag="y", name=f"yps{h}{qt}")
                dps = psd.tile([1, QT], F32, tag="d", name=f"dps{h}{qt}")
                a_tiles = [None] * nkc

                def emit_st_exp(kc):
                    # diagonal chunks only need q >= kc*128: slice width
                    o = kc - 4 * qt
                    f0 = max(o, 0) * 128  # in-tile q offset
                    sps = pst.tile([128, QT], F32, tag="st",
                                   name=f"sps{h}{qt}{kc}")
                    nc.tensor.matmul(
                        sps[:, f0:], qkT[:, 4 + h, kc * 128:(kc + 1) * 128],
                        qT_ap[:, f0:], start=True, stop=True,
                    )
                    if o >= 0:
                        nc.vector.tensor_add(
                            sps[:, f0:], sps[:, f0:],
                            masks_sb[:, :QT - f0])
                    a = patt.tile([128, QT], F16, tag="a", name=f"a{h}{qt}{kc}")
                    nc.scalar.activation(
                        a[:, f0:], sps[:, f0:],
                        mybir.ActivationFunctionType.Exp,
                        scale=SCALE,
                    )
                    a_tiles[kc] = (a, f0)

                emit_st_exp(0)
                if nkc > 1:
                    emit_st_exp(1)
                for kc in range(nkc):
                    if kc + 2 < nkc:
                        emit_st_exp(kc + 2)
                    a, f0 = a_tiles[kc]
                    nc.tensor.matmul(
                        yps[:, f0:], v_sb[:, kc, h * HD:(h + 1) * HD],
                        a[:, f0:],
                        start=(kc == 0), stop=(kc == nkc - 1),
                        skip_group_check=True,
                    )
                    nc.tensor.matmul(
                        dps[:, f0:], ones_sb, a[:, f0:],
                        start=(kc == 0), stop=(kc == nkc - 1),
                        skip_group_check=True,
                    )
                rinv = pnrm.tile([1, QT], F32, tag="rinv", name=f"ri{h}{qt}")
                nc.vector.reciprocal_approx_fast(rinv, dps)
                rb = pnrm.tile([128, QT], F32, tag="rb", name=f"rb{h}{qt}")
                nc.gpsimd.partition_broadcast(rb, rinv)
                nc.vector.tensor_mul(yT[:, h, q0:q0 + QT], yps, rb)
                if with_proj and qt > 0:
                    emit_proj_qt(qt - 1, ci)
            if with_proj:
                emit_proj_qt(NQT - 1, ci)

        emit_v()
        emit_qk(0)
        for h in range(NHL - 1):
            emit_qk(h + 1)
            emit_att(h)
        emit_att(NHL - 1, with_proj=True)

    nc.compile()
    return nc


def _prep_core_inputs(core, x16, W_attn, W_proj, cos2, sin2s, masks):
    b = core // 4
    g = core % 4
    heads = [g * NHL + i for i in range(NHL)]
    # stream_shuffle permutes within 32-partition blocks only: lay out each
    # block as [re pairs 16b..16b+15 | im pairs 16b..16b+15]
    perm = np.concatenate(
        [np.r_[2 * (16 * blk + np.arange(16)),
               2 * (16 * blk + np.arange(16)) + 1]
         for blk in range(4)]
    )

    xT = np.ascontiguousarray(x16[b].T)

    def qk_blocks(base):
        blocks = []
        for h in heads:
            blk = W_attn[:, base + h * HD: base + (h + 1) * HD][:, perm]
            blocks.append(blk.reshape(CC, 128, HD).transpose(1, 0, 2))
        return np.ascontiguousarray(np.stack(blocks, axis=0)).astype(np.float16)

    wq = qk_blocks(0)
    wk = qk_blocks(C)
    wv = np.concatenate(
        [W_attn[:, 2 * C + h * HD: 2 * C + (h + 1) * HD] for h in heads],
        axis=1,
    )  # (C, D_LOC)
    wv = np.ascontiguousarray(
        wv.reshape(CC, 128, D_LOC).transpose(1, 0, 2)).astype(np.float16)
    w2 = np.ascontiguousarray(
        np.stack([W_proj[h * HD:(h + 1) * HD, :] for h in heads], axis=0)
        .transpose(1, 0, 2)
    ).astype(np.float16)
    return {
        "xT": xT, "wq": wq, "wk": wk, "wv": wv, "w2": w2,
        "cos2": cos2, "sin2s": sin2s, "masks": masks,
    }


def _run(inputs, trace=False):
    global _compiled
    x = np.asarray(inputs["x"], dtype=np.float32)
    W_attn = np.asarray(inputs["W_attn"], dtype=np.float32)
    W_proj = np.asarray(inputs["W_proj"], dtype=np.float32)
    fc = np.asarray(inputs["freqs_cos"], dtype=np.float32)
    fs = np.asarray(inputs["freqs_sin"], dtype=np.float32)

    x16 = x.astype(np.float16)

    cosT = np.ascontiguousarray(fc.T)            # (64, T)
    sinT = np.ascontiguousarray(fs.T)
    # per 32-partition block b: partitions [0:16] carry cos/sin of pairs
    # 16b..16b+15 (re half, +sin), [16:32] the same freqs (im half, -sin)
    cos2 = np.concatenate(
        [np.concatenate([cosT[16 * blk:16 * (blk + 1)]] * 2, axis=0)
         for blk in range(4)], axis=0)           # (128, T)
    sin2s = np.concatenate(
        [np.concatenate([sinT[16 * blk:16 * (blk + 1)],
                         -sinT[16 * blk:16 * (blk + 1)]], axis=0)
         for blk in range(4)], axis=0)
    cos2 = np.ascontiguousarray(cos2).astype(np.float16)
    sin2s = np.ascontiguousarray(sin2s).astype(np.float16)

    # diagonal-chunk mask: within a chunk's valid q window, col j (q =
    # key_base + j): visible iff j >= partition k; cols >= 128 always visible
    ki = np.arange(128)[:, None]
    u = np.arange(QT)[None, :]
    masks = np.ascontiguousarray(
        np.where(u >= ki, 0.0, NEG).astype(np.float16))  # (128, 512)

    if _compiled is None:
        _compiled = _build()
    nc = _compiled

    in_maps = [
        _prep_core_inputs(c, x16, W_attn, W_proj, cos2, sin2s, masks)
        for c in range(N_CORES)
    ]
    res = run_bass_kernel_spmd(
        nc, in_maps, core_ids=list(range(N_CORES)), trace=trace)

    out = np.zeros((B, T, C), dtype=np.float32)
    for c in range(N_CORES):
        out[c // 4] += res.results[c]["out"].astype(np.float32)
    return out, res


def kernel(**inputs) -> np.ndarray:
    out, _ = _run(inputs, trace=False)
    return out
